# revision 47
# baseline (speedup 1.0000x reference)
"""Trainium2 Bass kernel for a 2-layer GAT + global-mean-pool + linear head.

Strategy (8 NeuronCores, SPMD):
  - Nodes are partitioned across cores by DESTINATION; each core owns all
    incoming edges of its 6250 nodes.  Per core, dsts are degree-sorted and
    bucketed into groups of 128 (one SBUF partition per dst); each dst's
    incoming edges occupy "slots" along the free dimension, padded to the
    group max (SPMD-shared across cores).
  - Layer features live in per-core HBM tables replicated via AllGather:
      table1 row (bf16): [h1 (H*HID) | as (H f32 bits) | ad (H f32 bits) | pad]
      table2 row (f32):  [h2 (OUT) | as2 | ad2 | pad]
    The per-edge "gather h[src]" is one dma_gather per (group, src-half)
    (int16 indices limit a gather table to 32768 rows, so the virtual node
    space is split in half: cores 0-3 = lo, 4-7 = hi).
  - Attention logits e = as[src]+ad[dst] are computed on the gathered rows
    (as rides inside the row; ad is a per-partition scalar), LeakyReLU via
    max(e, 0.2e), exp on the Scalar engine with accum_out giving the
    softmax denominator for free.  exp(e) * h multiplied per head with a
    stride-0 broadcast AP, then a pairwise tree-add reduces the slot axis.
  - Softmax max-subtraction is skipped (alpha = exp(e)/sum exp(e) is exact
    without it; logits are O(1) here so there is no overflow risk).
  - Pad slots gather a dedicated all-zero row whose "as" field is -88, so
    exp contributions are ~1e-38..1e-8 and no masking is needed.
  - log_softmax + per-graph mean pooling (one-hot matmul) + final linear
    run on-device; partial pooled sums are AllReduced.

Host-side work is limited to input prep: index/bucket construction from the
graph, weight folding (a_src/a_dst contracted into W), transposes and
replication of small parameters.
"""

import os
import ml_dtypes
import numpy as np

# Problem constants (from the problem spec; the harness always calls with
# these shapes).
N0, E0, G0 = 50000, 800000, 64
IN_DIM, HID0, OUT0, HEADS0 = 128, 64, 32, 4
NEG_SLOPE = 0.2
NCORES = 8


def _cfg(N, E, G, HID, OUT, H):
    NPC = N // NCORES
    NGRP = (NPC + 127) // 128
    NPCP = NGRP * 128
    NV = NCORES * NPCP
    HALF = NV // 2
    HH = H * HID
    # bf16 slots: h1 | as (H f32 -> 2H slots) | ad (2H slots); pad to 128-slot
    ROW1 = ((HH + 4 * H + 127) // 128) * 128
    ROW2 = (((OUT + 2) * 4 + 255) // 256) * 64  # f32 elems, 256B multiple
    return dict(N=N, E=E, G=G, HID=HID, OUT=OUT, H=H, NPC=NPC, NGRP=NGRP,
                NPCP=NPCP, NV=NV, HALF=HALF, HH=HH, ROW1=ROW1, ROW2=ROW2)


# --------------------------------------------------------------------------
# Host-side graph preprocessing
# --------------------------------------------------------------------------

def _prep(adj, batch, cfg):
    """Bucketed edge layout + all static per-core arrays.

    Single gather table covering all NV virtual rows: int16 gather indices
    are SIGNED, so with the gather base placed at row BASE0 = NV - 32768
    the offsets vid - BASE0 span [-32768, 32767] and one dma_gather per
    group covers every source node (the Q7 ucode computes addresses with
    an unsigned*signed multiply-accumulate, so negative offsets address
    rows below the base).  Only the FINAL index of each gather stream must
    be non-negative (the ucode strips trailing negatives as padding).
    """
    N, E = cfg["N"], cfg["E"]
    NPC, NGRP, NPCP, NV = (cfg[k] for k in ("NPC", "NGRP", "NPCP", "NV"))
    G = cfg["G"]
    BASE0 = max(0, NV - 32768)

    src = np.concatenate([np.asarray(adj[0]), np.arange(N)]).astype(np.int64)
    dst = np.concatenate([np.asarray(adj[1]), np.arange(N)]).astype(np.int64)
    EE = src.shape[0]

    core = dst // NPC
    dloc = dst % NPC
    deg = np.bincount(core * NPC + dloc, minlength=NCORES * NPC).reshape(NCORES, NPC)

    order = np.argsort(-deg, axis=1, kind="stable")          # [NC, NPC]
    pos = np.empty_like(order)
    np.put_along_axis(pos, order, np.broadcast_to(np.arange(NPC), (NCORES, NPC)), axis=1)

    nodes = np.arange(N)
    vid_of = (nodes // NPC) * NPCP + pos[nodes // NPC, nodes % NPC]

    vd = core * NPCP + pos[core, dloc]
    vs = vid_of[src]

    dall = np.bincount(vd, minlength=NV)

    # slot rank of each edge within its vd bucket
    ordE = np.argsort(vd, kind="stable")
    ks = vd[ordE]
    starts = np.r_[0, np.flatnonzero(np.diff(ks)) + 1]
    gid = np.zeros(EE, np.int64)
    gid[starts[1:]] = 1
    gid = np.cumsum(gid)
    rank_sorted = np.arange(EE) - starts[gid]
    rank = np.empty(EE, np.int64)
    rank[ordE] = rank_sorted

    gi = np.arange(NV) % NPCP // 128                          # group of each vid
    kall = np.zeros(NGRP, np.int64)
    np.maximum.at(kall, gi, dall)
    kall = np.maximum(kall, 1)

    # pad target: an all-(-88) padded-dst row with vid >= BASE0 so its
    # gather offset is non-negative (never stripped as trailing padding).
    if NPCP > NPC:
        cstar = min(max(0, -(-(BASE0 - NPC) // NPCP)), NCORES - 1)
        vpad = cstar * NPCP + NPC
        assert vpad >= BASE0
    else:
        vpad = BASE0       # tiny test configs only

    def build_idx(kall):
        offs = np.r_[0, np.cumsum(128 * kall)]
        C = int(offs[-1])
        idx = np.full((NCORES, C), vpad - BASE0, np.int64)
        ec = vd // NPCP                            # owning core of each edge
        eg = (vd % NPCP) // 128                    # group
        ep = vd % 128                              # partition
        addr = offs[eg] + rank * 128 + ep
        idx[ec, addr] = vs - BASE0
        return offs, idx

    offs, idx = build_idx(kall)
    assert idx.min() >= -32768 and idx.max() <= 32767

    # The ucode strips trailing negative indices; make sure the last stream
    # element of every (core, group) gather is non-negative by swapping
    # within the partition-127 dst's slot list (slot order is arbitrary).
    # If that dst's slots are all negative, grow the group by one pad column.
    grow = set()
    for c in range(NCORES):
        for g in range(NGRP):
            k = int(kall[g])
            tail = int(offs[g]) + (k - 1) * 128 + 127
            if idx[c, tail] >= 0:
                continue
            slots = int(offs[g]) + np.arange(k) * 128 + 127
            ok = np.nonzero(idx[c, slots] >= 0)[0]
            if ok.size == 0:
                grow.add(g)
                continue
            r = int(ok[0])
            idx[c, slots[-1]], idx[c, slots[r]] = idx[c, slots[r]], idx[c, slots[-1]]
    if grow:
        for g in grow:
            kall[g] += 1
        offs, idx = build_idx(kall)
        for c in range(NCORES):
            for g in range(NGRP):
                k = int(kall[g])
                tail = int(offs[g]) + (k - 1) * 128 + 127
                if idx[c, tail] < 0:
                    slots = int(offs[g]) + np.arange(k) * 128 + 127
                    ok = np.nonzero(idx[c, slots] >= 0)[0]
                    r = int(ok[0])
                    idx[c, slots[-1]], idx[c, slots[r]] = (
                        idx[c, slots[r]], idx[c, slots[-1]])

    def pack16(a):  # stream position i -> partition i%16, col i//16.
        # CoreSim reads the idx AP at partitions 0..15; the Q7 ucode for
        # queue q reads a 16-partition block whose base scales with q --
        # replicate across all eight 16-partition blocks so any queue works.
        L = a.shape[1]
        p = np.zeros((a.shape[0], 128, L // 16), np.int16)
        w = a.reshape(a.shape[0], L // 16, 16).transpose(0, 2, 1)
        for j in range(8):
            p[:, 16 * j:16 * (j + 1), :] = w
        return p

    # pooling one-hot + counts
    batch = np.asarray(batch).astype(np.int64)
    Mpool = np.zeros((NCORES, NPCP, G), np.float32)
    for c in range(NCORES):
        ns = nodes[nodes // NPC == c]
        Mpool[c, pos[c, ns % NPC], batch[ns]] = 1.0
    counts = np.bincount(batch, minlength=G).astype(np.float32)
    inv_counts = 1.0 / np.maximum(counts, 1.0)

    perm = np.empty(NV, np.int64)        # vid -> original node (or -1 pad)
    perm.fill(-1)
    perm[vid_of] = nodes

    return dict(idx=pack16(idx), kall=kall, offs=offs, BASE0=BASE0,
                vid_of=vid_of, perm=perm, Mpool=Mpool, inv_counts=inv_counts,
                vs=vs, vd=vd)


def _fold_weights(W1, a1_src, a1_dst, W2, a2_src, a2_dst, cfg):
    H, HID, OUT, HH = cfg["H"], cfg["HID"], cfg["OUT"], cfg["HH"]
    Ws = np.stack([W1[:, h * HID:(h + 1) * HID] @ a1_src[h] for h in range(H)], 1)
    Wd = np.stack([W1[:, h * HID:(h + 1) * HID] @ a1_dst[h] for h in range(H)], 1)
    Waug1 = np.concatenate([W1, Ws, Wd], 1).astype(np.float32)      # [IN, HH+2H]
    Waug2 = np.concatenate([W2, W2 @ a2_src[0][:, None], W2 @ a2_dst[0][:, None]],
                           1).astype(np.float32)                     # [HH, OUT+2]
    return Waug1, Waug2


# --------------------------------------------------------------------------
# Bass program
# --------------------------------------------------------------------------

def _build_program(cfg, prep):
    import concourse.bass as bass
    import concourse.bacc as bacc
    import concourse.mybir as mybir
    import concourse.tile as tile
    from concourse.bass import AP

    dt = mybir.dt
    Alu = mybir.AluOpType
    Act = mybir.ActivationFunctionType

    H, HID, OUT, HH = cfg["H"], cfg["HID"], cfg["OUT"], cfg["HH"]
    NGRP, NPCP, NV = cfg["NGRP"], cfg["NPCP"], cfg["NV"]
    ROW1, ROW2, G = cfg["ROW1"], cfg["ROW2"], cfg["G"]
    NPC = cfg["NPC"]
    kall, offs, BASE0 = prep["kall"], prep["offs"], prep["BASE0"]
    C = int(offs[-1])
    W1C = HH + 2 * H

    def bcast(ap, n):
        """Append a stride-0 inner dim of size n to an AP."""
        return AP(ap.tensor, ap.offset, list(ap.ap) + [[0, n]])

    _regcache = {}

    from concourse import library_config
    import os as _os
    PHASES = int(_os.environ.get("GAT_PHASES", "9"))
    NQ = int(_os.environ.get("GAT_QUEUES", "4"))
    # dma_gather's descriptor generation runs on the Q7 core pair
    # cpu_id/2 == queue_num, so spreading gathers across the 4 SWDGE
    # queues parallelizes descriptor generation 4-way.
    nc = bacc.Bacc(None, target_bir_lowering=False, num_swdge_queues=NQ,
                   dynamic_dma_scratch_size=32768)

    def reg_of(v):
        # gpsimd registers are a scarce pool; reuse one per distinct constant
        if v not in _regcache:
            _regcache[v] = nc.gpsimd.to_reg(v)
        return _regcache[v]

    # ---- inputs
    xT = nc.dram_tensor("xT", [IN_DIM, NPCP], dt.float32, kind="ExternalInput")
    Waug1 = nc.dram_tensor("Waug1", [IN_DIM, W1C], dt.float32, kind="ExternalInput")
    Waug2 = nc.dram_tensor("Waug2", [HH, OUT + 2], dt.float32, kind="ExternalInput")
    idx_d = nc.dram_tensor("idx", [128, C // 16], dt.int16, kind="ExternalInput")
    Mpool_d = nc.dram_tensor("Mpool", [NPCP, G], dt.bfloat16, kind="ExternalInput")
    b1rep = nc.dram_tensor("b1rep", [128, HH], dt.float32, kind="ExternalInput")
    b2rep = nc.dram_tensor("b2rep", [128, OUT], dt.float32, kind="ExternalInput")
    invc_d = nc.dram_tensor("invc", [G, 1], dt.float32, kind="ExternalInput")
    linW_d = nc.dram_tensor("linW", [128, OUT], dt.float32, kind="ExternalInput")
    linb_d = nc.dram_tensor("linb", [G, 1], dt.float32, kind="ExternalInput")
    sw_d = nc.dram_tensor("swrep", [G, 1], dt.float32, kind="ExternalInput")
    ident_d = nc.dram_tensor("ident", [128, 128], dt.float32, kind="ExternalInput")
    npad = NPCP - NPC
    padfix_d = (nc.dram_tensor("padfix", [max(npad, 1), 2 * H + 1], dt.float32,
                               kind="ExternalInput"))
    out_d = nc.dram_tensor("out", [G, 1], dt.float32, kind="ExternalOutput")

    LINEARIZE = _os.environ.get("GAT_LINEARIZE", "0") == "1"
    with tile.TileContext(nc, linearize=LINEARIZE) as tc:
        with (
            tc.tile_pool(name="dram", bufs=1, space="DRAM") as dram,
            tc.tile_pool(name="const", bufs=1) as cpool,
            tc.tile_pool(name="stage", bufs=3) as spool,
            tc.tile_pool(name="psum", bufs=2, space="PSUM") as psum,
            tc.tile_pool(name="psumb", bufs=1, space="PSUM") as psumb,
            tc.tile_pool(name="pacc", bufs=1, space="PSUM") as pacc,
            tc.tile_pool(name="gat", bufs=2) as gpool,
            tc.tile_pool(name="msg", bufs=1) as mpool,
            tc.tile_pool(name="msg2", bufs=2) as mpool2,
            tc.tile_pool(name="eph", bufs=2) as epool,
            tc.tile_pool(name="persist", bufs=1) as ppool,
        ):
            f32, bf16 = dt.float32, dt.bfloat16
            # dma_gather/dma_scatter_add live in the 'mlp' GPSIMD library;
            # load it before any extended Pool instruction executes.
            nc.gpsimd.load_library(library_config.mlp)
            slice1 = dram.tile([NPCP, ROW1], bf16, tag="slice1")
            table1 = nc.dram_tensor("table1", [NV, ROW1], bf16,
                                    addr_space="Shared")
            slice2 = dram.tile([NPCP, ROW2], f32, tag="slice2")
            table2 = nc.dram_tensor("table2", [NV, ROW2], f32,
                                    addr_space="Shared")
            ar_in = dram.tile([G, 1], f32, tag="ar_in")
            ar_out = dram.tile([G, 1], f32, tag="ar_out")

            # ---- constants in SBUF
            W1_sb = cpool.tile([128, W1C], f32, tag="W1")
            nc.sync.dma_start(W1_sb[:], Waug1[:])
            W2_sb = cpool.tile([128, (HH // 128) * (OUT + 2)], bf16, tag="W2")
            W2v = W2_sb[:].rearrange("p (b c) -> p b c", c=OUT + 2)
            for b in range(HH // 128):
                nc.gpsimd.dma_start(W2v[:, b, :], Waug2[b * 128:(b + 1) * 128, :])
            ident_sb = cpool.tile([128, 128], f32, tag="ident")
            nc.sync.dma_start(ident_sb[:], ident_d[:])
            identb = cpool.tile([128, 128], bf16, tag="identb")
            nc.vector.tensor_copy(identb[:], ident_sb[:])
            b1_sb = cpool.tile([128, HH], f32, tag="b1")
            nc.sync.dma_start(b1_sb[:], b1rep[:])
            b2_sb = cpool.tile([128, OUT], f32, tag="b2")
            nc.sync.dma_start(b2_sb[:], b2rep[:])
            idx_sb = cpool.tile([128, C // 16], dt.int16, tag="idx")
            nc.sync.dma_start(idx_sb[:], idx_d[:])
            Mp_sb = cpool.tile([128, NGRP * G], bf16, tag="Mp")
            Mpv = Mp_sb[:].rearrange("p (g c) -> p g c", c=G)
            Mdv = Mpool_d[:].rearrange("(g p) c -> p g c", p=128)
            nc.sync.dma_start(Mpv[:], Mdv[:])
            linW_sb = cpool.tile([128, OUT], f32, tag="linW")
            nc.sync.dma_start(linW_sb[:], linW_d[:])
            invc_sb = cpool.tile([G, 1], f32, tag="invc")
            nc.sync.dma_start(invc_sb[:], invc_d[:])
            linb_sb = cpool.tile([G, 1], f32, tag="linb")
            nc.sync.dma_start(linb_sb[:], linb_d[:])
            sw_sb = cpool.tile([G, 1], f32, tag="swrep")
            nc.sync.dma_start(sw_sb[:], sw_d[:])

            # ---- P1: slice1 = [x@W1 | as | ad] for own nodes
            s1f32 = slice1[:].bitcast(f32)   # [NPCP, ROW1//2] f32 view
            pad1 = ROW1 - (HH + 4 * H)
            pad2 = ROW2 - (OUT + 2)
            as_st = ppool.tile([128, NGRP * 2 * H], f32, tag="as_st")
            asv = as_st[:].rearrange("p (g c) -> p g c", c=2 * H)
            zpad1 = cpool.tile([128, max(pad1, 1)], bf16, tag="zpad1")
            nc.vector.memset(zpad1[:], 0.0)
            zpad2 = cpool.tile([128, max(pad2, 1)], f32, tag="zpad2")
            nc.vector.memset(zpad2[:], 0.0)
            for t in range(NGRP if PHASES >= 1 else 0):
                xt_t = spool.tile([128, 128], f32, tag="xt")
                nc.sync.dma_start(xt_t[:], xT[:, t * 128:(t + 1) * 128])
                ps = psum.tile([128, W1C], f32, tag="ps1")
                nc.tensor.matmul(ps[:], xt_t[:], W1_sb[:], start=True, stop=True)
                st_h = spool.tile([128, HH], bf16, tag="st_h")
                nc.scalar.activation(st_h[:], ps[:, :HH], Act.Copy)
                nc.vector.tensor_copy(asv[:, t, :], ps[:, HH:])
                nc.sync.dma_start(slice1[t * 128:(t + 1) * 128, :HH], st_h[:])
                if pad1 > 0:
                    nc.sync.dma_start(
                        slice1[t * 128:(t + 1) * 128, HH + 4 * H:], zpad1[:])
            if PHASES >= 1:
                nc.sync.dma_start(
                    s1f32[:, HH // 2:HH // 2 + 2 * H]
                    .rearrange("(g p) c -> p g c", p=128), asv[:])
            if npad > 0:
                nc.sync.dma_start(
                    s1f32[NPC:NPCP, HH // 2:HH // 2 + 2 * H],
                    padfix_d[:, :2 * H])

            if PHASES >= 2:
                # ---- P2: AllGather table1
                nc.gpsimd.collective_compute(
                    "AllGather", Alu.bypass,
                    replica_groups=[list(range(NCORES))],
                    ins=[slice1.opt()], outs=[table1[:]])

            # ---- persistent accumulators
            dn_all = ppool.tile([128, NGRP * H], f32, tag="dn")
            o1_all = ppool.tile([128, NGRP * HH], bf16, tag="o1")
            adv = asv[:, :, H:2 * H]         # per-dst a_dst·h, straight from P1

            # Pack adjacent groups into shared gather calls, bounded by the
            # largest single group so the staging tile size is unchanged.
            # Fewer dma_gather calls -> less fixed SWDGE/Q7 overhead.
            B = int(kall.max())
            packs = []
            cur, cursum = [], 0
            for g in range(NGRP):
                if cursum + int(kall[g]) > B and cur:
                    packs.append(cur)
                    cur, cursum = [], 0
                cur.append(g)
                cursum += int(kall[g])
            packs.append(cur)

            # ---- P3: layer-1 message passing
            def p3_group(g, K, Gv, Gf):
                Ef = epool.tile([128, H * K], f32, tag="E1")
                for h in range(H):
                    nc.vector.tensor_scalar_add(
                        Ef[:, h * K:(h + 1) * K], Gf[:, :, HH // 2 + h],
                        adv[:, g, h:h + 1])
                Et = epool.tile([128, H * K], f32, tag="E1t")
                nc.vector.tensor_scalar_mul(Et[:], Ef[:], NEG_SLOPE)
                nc.vector.tensor_tensor(Ef[:], Ef[:], Et[:], op=Alu.max)
                exb = epool.tile([128, H * K], bf16, tag="exb")
                for h in range(H):
                    nc.scalar.activation(
                        exb[:, h * K:(h + 1) * K], Ef[:, h * K:(h + 1) * K],
                        Act.Exp, accum_out=dn_all[:, g * H + h:g * H + h + 1])
                mm = mpool.tile([128, K * HH], bf16, tag="mm")
                mv = mm[:].rearrange("p (k f) -> p k f", f=HH)
                for h in range(H):
                    nc.vector.tensor_tensor(
                        mv[:, :, h * HID:(h + 1) * HID],
                        Gv[:, :, h * HID:(h + 1) * HID],
                        bcast(exb[:, h * K:(h + 1) * K], HID), op=Alu.mult)
                cur = K
                while cur > 1:
                    half = cur // 2
                    nc.vector.tensor_tensor(
                        mv[:, :half, :], mv[:, :half, :],
                        mv[:, half:2 * half, :], op=Alu.add)
                    if cur % 2:
                        nc.vector.tensor_tensor(
                            mv[:, 0, :], mv[:, 0, :], mv[:, cur - 1, :],
                            op=Alu.add)
                    cur = half
                rdn = epool.tile([128, H], f32, tag="rdn")
                nc.vector.reciprocal(rdn[:], dn_all[:, g * H:(g + 1) * H])
                o1g = o1_all[:, g * HH:(g + 1) * HH]
                for h in range(H):
                    nc.vector.tensor_scalar_mul(
                        o1g[:, h * HID:(h + 1) * HID],
                        mv[:, 0, h * HID:(h + 1) * HID], rdn[:, h:h + 1])
                nc.vector.tensor_tensor(o1g, o1g, b1_sb[:], op=Alu.add)
                nc.vector.tensor_scalar_max(o1g, o1g, 0.0)

            for pi, pack in enumerate(packs if PHASES >= 3 else []):
                KP = int(sum(kall[g] for g in pack))
                g0, g1 = pack[0], pack[-1]
                Gt = gpool.tile([128, B * ROW1], bf16, tag="G1")
                Gtv = Gt[:].rearrange("p (k r) -> p k r", r=ROW1)
                nc.gpsimd.dma_gather(
                    Gtv[:, :KP, :], table1[BASE0:NV, :],
                    idx_sb[:, int(offs[g0]) // 16:int(offs[g1 + 1]) // 16],
                    128 * KP, reg_of(128 * KP), ROW1, single_packet=False,
                    queue_num=pi % NQ)
                Gtf = Gt[:].bitcast(f32).rearrange("p (k r) -> p k r", r=ROW1 // 2)
                ck = 0
                for g in pack:
                    K = int(kall[g])
                    p3_group(g, K, Gtv[:, ck:ck + K, :], Gtf[:, ck:ck + K, :])
                    ck += K

            # ---- P4: slice2 = [relu(o1) @ W2 | as2 | ad2]
            if PHASES < 3:
                nc.vector.memset(dn_all[:], 1.0)
                nc.vector.memset(o1_all[:], 0.0)
            s2v = slice2[:].rearrange("(g p) r -> g p r", p=128)
            st2_st = ppool.tile([128, NGRP * (OUT + 2)], f32, tag="st2st")
            st2v = st2_st[:].rearrange("p (g c) -> p g c", c=OUT + 2)
            for t in range(NGRP if PHASES >= 4 else 0):
                ps2 = psumb.tile([128, OUT + 2], f32, tag="ps2")
                for b in range(HH // 128):
                    pst = psum.tile([128, 128], bf16, tag="pst")
                    nc.tensor.transpose(
                        pst[:], o1_all[:, t * HH + b * 128:t * HH + (b + 1) * 128],
                        identb[:])
                    sbt = spool.tile([128, 128], bf16, tag="sbt")
                    nc.scalar.activation(sbt[:], pst[:], Act.Copy)
                    nc.tensor.matmul(ps2[:], sbt[:], W2v[:, b, :],
                                     start=(b == 0), stop=(b == HH // 128 - 1))
                nc.scalar.activation(st2v[:, t, :], ps2[:], Act.Copy)
            if PHASES >= 4:
                nc.sync.dma_start(
                    s2v[:, :, :OUT + 2].rearrange("g p c -> p g c"), st2v[:])
                if pad2 > 0:
                    for t in range(NGRP):
                        nc.sync.dma_start(s2v[t, :, OUT + 2:], zpad2[:])
            if npad > 0:
                nc.sync.dma_start(slice2[NPC:NPCP, OUT:OUT + 1],
                                  padfix_d[:, 2 * H:2 * H + 1])

            # ---- P5: AllGather table2
            if PHASES >= 5:
                nc.gpsimd.collective_compute(
                    "AllGather", Alu.bypass,
                    replica_groups=[list(range(NCORES))],
                    ins=[slice2.opt()], outs=[table2[:]])

            ad2v = st2v[:, :, OUT + 1:OUT + 2]   # [128, NGRP, 1]

            pspool = pacc.tile([G, 1], f32, tag="pspool")
            pspool_s = pacc.tile([G, 1], f32, tag="pspools")
            o2_all = ppool.tile([128, NGRP * OUT], f32, tag="o2all")
            mx_all = ppool.tile([128, NGRP], f32, tag="mxall")
            nmx_all = ppool.tile([128, NGRP], f32, tag="nmxall")
            se_all = ppool.tile([128, NGRP], f32, tag="seall")

            if PHASES < 6:
                zmm = epool.tile([128, G], bf16, tag="zmm")
                nc.vector.memset(zmm[:], 0.0)
                zm2 = epool.tile([128, 1], bf16, tag="zm2")
                nc.vector.memset(zm2[:], 0.0)
                nc.tensor.matmul(pspool[:], zmm[:], zm2[:],
                                 start=True, stop=True)
                nc.tensor.matmul(pspool_s[:], zmm[:], zm2[:],
                                 start=True, stop=True)
            # ---- P6: layer-2 message passing + log_softmax + pooling
            def p6_group(g, K, G2v):
                E2 = epool.tile([128, K], f32, tag="E2")
                nc.vector.tensor_scalar_add(E2[:], G2v[:, :, OUT],
                                            ad2v[:, g, :])
                E2t = epool.tile([128, K], f32, tag="E2t")
                nc.vector.tensor_scalar_mul(E2t[:], E2[:], NEG_SLOPE)
                nc.vector.tensor_tensor(E2[:], E2[:], E2t[:], op=Alu.max)
                ex2 = epool.tile([128, K], f32, tag="ex2")
                dn2 = epool.tile([128, 1], f32, tag="dn2")
                nc.scalar.activation(ex2[:], E2[:], Act.Exp, accum_out=dn2[:])
                mm2 = mpool.tile([128, K * OUT], f32, tag="mm2")
                m2v = mm2[:].rearrange("p (k f) -> p k f", f=OUT)
                nc.vector.tensor_tensor(m2v[:], G2v[:, :, :OUT],
                                        bcast(ex2[:], OUT), op=Alu.mult)
                cur = K
                while cur > 1:
                    half = cur // 2
                    nc.vector.tensor_tensor(m2v[:, :half, :], m2v[:, :half, :],
                                            m2v[:, half:2 * half, :], op=Alu.add)
                    if cur % 2:
                        nc.vector.tensor_tensor(m2v[:, 0, :], m2v[:, 0, :],
                                                m2v[:, cur - 1, :], op=Alu.add)
                    cur = half
                rdn2 = epool.tile([128, 1], f32, tag="rdn2")
                nc.vector.reciprocal(rdn2[:], dn2[:])
                o2 = o2_all[:, g * OUT:(g + 1) * OUT]
                nc.vector.tensor_scalar_mul(o2, m2v[:, 0, :], rdn2[:])
                nc.vector.tensor_tensor(o2, o2, b2_sb[:], op=Alu.add)
                nc.vector.tensor_reduce(mx_all[:, g:g + 1], o2,
                                        axis=mybir.AxisListType.X, op=Alu.max)
                nc.vector.tensor_scalar_mul(nmx_all[:, g:g + 1],
                                            mx_all[:, g:g + 1], -1.0)
                sexp = epool.tile([128, OUT], f32, tag="sexp")
                nc.scalar.activation(sexp[:], o2, Act.Exp,
                                     bias=nmx_all[:, g:g + 1],
                                     accum_out=se_all[:, g:g + 1])
                # pool(lsb@linW) == pool(o2@linW) + sum(linW)*pool(shift);
                # accumulate the o2@linW part here, the shift part in P6b.
                lw = epool.tile([128, OUT], f32, tag="lw")
                nc.vector.tensor_tensor(lw[:], o2, linW_sb[:], op=Alu.mult)
                q32 = epool.tile([128, 1], f32, tag="q32")
                nc.vector.tensor_reduce(q32[:], lw[:],
                                        axis=mybir.AxisListType.X, op=Alu.add)
                qb = epool.tile([128, 1], bf16, tag="qb")
                nc.vector.tensor_copy(qb[:], q32[:])
                nc.tensor.matmul(pspool[:], Mpv[:, g, :], qb[:],
                                 start=(g == 0), stop=(g == NGRP - 1))

            for pi, pack in enumerate(packs if PHASES >= 6 else []):
                KP = int(sum(kall[g] for g in pack))
                g0, g1 = pack[0], pack[-1]
                G2 = gpool.tile([128, B * ROW2], f32, tag="G2")
                G2tv = G2[:].rearrange("p (k r) -> p k r", r=ROW2)
                nc.gpsimd.dma_gather(
                    G2tv[:, :KP, :], table2[BASE0:NV, :],
                    idx_sb[:, int(offs[g0]) // 16:int(offs[g1 + 1]) // 16],
                    128 * KP, reg_of(128 * KP), ROW2, single_packet=False,
                    queue_num=pi % NQ)
                ck = 0
                for g in pack:
                    K = int(kall[g])
                    p6_group(g, K, G2tv[:, ck:ck + K, :])
                    ck += K

            # ---- P6b: batched log-sum-exp + shift pooling
            if PHASES >= 6:
                lse_all = ppool.tile([128, NGRP], f32, tag="lseall")
                nc.scalar.activation(lse_all[:], se_all[:], Act.Ln)
                shift_bf = ppool.tile([128, NGRP], bf16, tag="shiftbf")
                nc.vector.tensor_tensor(shift_bf[:], nmx_all[:], lse_all[:],
                                        op=Alu.subtract)
                for g in range(NGRP):
                    nc.tensor.matmul(pspool_s[:], Mpv[:, g, :],
                                     shift_bf[:, g:g + 1],
                                     start=(g == 0), stop=(g == NGRP - 1))

            # ---- P7: AllReduce pooled sums, mean + bias
            NOTAIL = _os.environ.get("GAT_NOTAIL", "0") == "1"
            qs = spool.tile([G, 1], f32, tag="qs")
            nc.vector.tensor_scalar_mul(qs[:], pspool_s[:], sw_sb[:])
            pool_sb = spool.tile([G, 1], f32, tag="pool")
            nc.vector.tensor_tensor(pool_sb[:], qs[:], pspool[:], op=Alu.add)
            nc.sync.dma_start(ar_in[:], pool_sb[:])
            if not NOTAIL:
                nc.gpsimd.collective_compute(
                    "AllReduce", Alu.add,
                    replica_groups=[list(range(NCORES))],
                    ins=[ar_in.opt()], outs=[ar_out.opt()])
            else:
                nc.sync.dma_start(ar_out[:], ar_in[:])
            pool2 = spool.tile([G, 1], f32, tag="pool2")
            nc.sync.dma_start(pool2[:], ar_out[:])
            fin = spool.tile([G, 1], f32, tag="fin")
            nc.vector.tensor_scalar(fin[:], pool2[:], invc_sb[:], linb_sb[:],
                                    op0=Alu.mult, op1=Alu.add)
            nc.sync.dma_start(out_d[:], fin[:])

    nc.compile()
    return nc


# --------------------------------------------------------------------------
# Input map construction + entry point
# --------------------------------------------------------------------------

def _in_maps(inputs, cfg, prep):
    x = np.asarray(inputs["x"], np.float32)
    Waug1, Waug2 = _fold_weights(
        np.asarray(inputs["W1"], np.float32), np.asarray(inputs["a1_src"], np.float32),
        np.asarray(inputs["a1_dst"], np.float32), np.asarray(inputs["W2"], np.float32),
        np.asarray(inputs["a2_src"], np.float32), np.asarray(inputs["a2_dst"], np.float32),
        cfg)
    H, HH, OUT, G = cfg["H"], cfg["HH"], cfg["OUT"], cfg["G"]
    NPC, NPCP = cfg["NPC"], cfg["NPCP"]
    npad = NPCP - NPC
    b1 = np.asarray(inputs["b1"], np.float32)
    b2 = np.asarray(inputs["b2"], np.float32)
    b1rep = np.broadcast_to(b1, (128, HH)).copy()
    b2rep = np.broadcast_to(b2, (128, OUT)).copy()
    invc = prep["inv_counts"].reshape(G, 1).astype(np.float32)
    linW_vec = np.asarray(inputs["lin_W"], np.float32).ravel()
    linW = np.ascontiguousarray(np.broadcast_to(linW_vec, (128, OUT)))
    swrep = np.full((G, 1), linW_vec.sum(), np.float32)
    linb = np.broadcast_to(np.asarray(inputs["lin_b"], np.float32), (G,)) \
        .reshape(G, 1).astype(np.float32).copy()
    ident = np.eye(128, dtype=np.float32)
    padfix = np.full((max(npad, 1), 2 * H + 1), -88.0, np.float32)

    maps = []
    for c in range(NCORES):
        vids = np.arange(c * NPCP, (c + 1) * NPCP)
        orig = prep["perm"][vids]
        xs = np.zeros((NPCP, IN_DIM), np.float32)
        real = orig >= 0
        xs[real] = x[orig[real]]
        maps.append(dict(
            xT=np.ascontiguousarray(xs.T), Waug1=Waug1, Waug2=Waug2,
            idx=prep["idx"][c],
            Mpool=prep["Mpool"][c].astype(ml_dtypes.bfloat16),
            b1rep=b1rep, b2rep=b2rep, invc=invc, linW=linW, linb=linb,
            swrep=swrep,
            ident=ident, padfix=padfix))
    return maps


def _run_hw(nc, maps):
    import time as _time
    from concourse.bass_utils import run_bass_kernel_spmd
    kw = {}
    if os.environ.get("GAT_TRACE", "0") == "1":
        kw = dict(trace=True,
                  trace_cores=[int(c) for c in
                               os.environ.get("GAT_TRACE_CORES", "0").split(",")])
    res = run_bass_kernel_spmd(nc, maps, list(range(NCORES)), **kw)
    if kw and res.exec_time_ns is not None:
        print("HW exec time: %d ns" % res.exec_time_ns)
        if res.instructions_and_trace:
            print("trace path:", res.instructions_and_trace[1])
    if os.environ.get("GAT_TIMEIT", "0") == "1":
        # repeat executions (NEFF cached) -> wall-time upper bound on HW time
        best = None
        for _ in range(3):
            t0 = _time.time()
            run_bass_kernel_spmd(nc, maps, list(range(NCORES)))
            dt_ = _time.time() - t0
            best = dt_ if best is None else min(best, dt_)
        print("HW exec time: %d ns (repeat-call wall time, upper bound)"
              % int(best * 1e9))
    return res.results[0]["out"]


def _run_sim(nc, maps):
    from concourse.bass_interp import MultiCoreSim
    # ignore_data_errors: as/ad ride as f32 bit-patterns inside bf16 tables,
    # which trips the sim's bf16 finite-checker (false alarm).
    sim = MultiCoreSim(nc, NCORES, ignore_data_errors=True)
    for c in range(NCORES):
        for k, v in maps[c].items():
            sim.cores[c].tensor(k)[:] = v
    sim.simulate()
    return np.array(sim.cores[0].tensor("out"))


def kernel_with_cfg(inputs, N, E, G, HID, OUT, H, mode="hw"):
    cfg = _cfg(N, E, G, HID, OUT, H)
    prep = _prep(inputs["adj"], inputs["batch"], cfg)
    maps = _in_maps(inputs, cfg, prep)
    nc = _build_program(cfg, prep)
    if mode == "sim":
        out = _run_sim(nc, maps)
    else:
        out = _run_hw(nc, maps)
    return np.asarray(out, np.float32)


def kernel(**inputs):
    mode = os.environ.get("GAT_KERNEL_MODE", "hw")
    return kernel_with_cfg(inputs, N0, E0, G0, HID0, OUT0, HEADS0, mode=mode)

